# revision 21
# baseline (speedup 1.0000x reference)
"""Trainium2 Bass kernel for nn_AdversarialLoss_PDD (pairwise JS-divergence loss).

Single fused kernel. Math (validated vs reference in fp64):
  raw = f @ W.T + b, y = raw/2, Ss/St = softmax(raw/4),
  H_i = sum_c S ln S, JS[i,j] = 0.5(H_i+H_j) + ln2 - 0.5*G[i,j],
  G[i,j] = sum_c (S_i+S_j) ln(S_i+S_j).

Only same-label (ss) and label==pseudo&conf (st) pairs contribute. The ss
pair list depends only on labels (known before launch), so rows are
assigned to cores BY CLASS: each core gets 64 source rows (same-label
groups co-located) + 64 target rows.  One kernel per core then:
  1. logits raw'' = fp8(f) @ fp8(W*sqrt(K)).T + sqrt(K)*b
     (8 fp8 DoubleRow matmuls, 2 contraction chunks each, + 1 bias outer)
  2. ET = exp(raw''/(4*sqrt(K))) bf16, z = rowsum, rz = 1/z
  3. U = matmul(E*rz, ET): one-hot pair-selection matrix E (host input)
     gives U[p,c] = S_a + S_b for pair p's rows (a,b)
  4. G[p] = sum_c U ln U  via ACT Ln + DVE mult-accum
Outputs: raw'' (bf16) and G (f32). Host computes softmax stats/H/conf/
pseudo from raw'', the ~35 st pairs + spilled ss pairs, masked means.
End-to-end loss rel err vs fp64 reference (on HW): 1.9e-3 (tol 2e-2).

Timing (CoreSim cost model, single launch): 7779 ns vs 20894 ns for the
previous two-phase windowed kernel (2.7x).  Remaining time is structural:
~2950 input DMA+matmul front (bus-bound at 0.53MB/core fp8), ~1900 the
exp->z->1/z->E'->matmul->ln->mult chain (sem-prop dominated), ~2200 the
final G DMA (seq+DGE+sem fixed costs), ~600 drain.  Tried and rejected:
DVE divide (not lowerable), partition-split pipelining (engine time is
free-dim bound), act-table tricks beyond the single-table pin, DMA
queue/group reshuffles (bus-bound floor reached).
"""

import math
import sys
import numpy as np
from contextlib import ExitStack

for _p in ("/opt/trn_rl_repo", "/root/.axon_site/_ro/trn_rl_repo"):
    if _p not in sys.path:
        sys.path.append(_p)

import ml_dtypes
import concourse.bass as bass
import concourse.tile as tile
from concourse import bacc, mybir
from concourse.bass_utils import run_bass_kernel_spmd

F32 = mybir.dt.float32
BF16 = mybir.dt.bfloat16
FP8 = mybir.dt.float8e4
AL = mybir.AluOpType
AF = mybir.ActivationFunctionType
NP_FP8 = ml_dtypes.float8_e4m3
NP_BF16 = ml_dtypes.bfloat16

NCORES = 8
C = 128            # n classes
K = 2048           # in features
N = 1024           # batch (source+target)
BS = 512           # source rows
SRC_PC = BS // NCORES   # 64 source slots per core
TGT_PC = BS // NCORES   # 64 target slots per core
RPC = SRC_PC + TGT_PC   # 128 rows per core
PCAP = 128              # pair columns per core (partition-limited)

THRESHOLD = 0.05
LN2 = math.log(2.0)
SC = math.sqrt(float(K))         # f8 weight pre-scale
EXPS = 0.25 / SC                 # device exp scale for tempered softmax

_cache = {}


def _build_fused():
    """Per core: 128-row logits (fp8 DoubleRow matmul) + per-pair G."""
    nc = bacc.Bacc(None, target_bir_lowering=False)
    fT4 = nc.dram_tensor("fT4", [2, 128, 8, RPC], FP8, kind="ExternalInput")
    WT4 = nc.dram_tensor("WT4", [2, 128, 8, C], FP8, kind="ExternalInput")
    EIN = nc.dram_tensor("EIN", [SRC_PC, PCAP], BF16, kind="ExternalInput")
    OB = nc.dram_tensor("OB", [1, RPC + C], BF16, kind="ExternalInput")
    YO = nc.dram_tensor("YO", [RPC, C], BF16, kind="ExternalOutput")
    GO = nc.dram_tensor("GO", [PCAP, 1], F32, kind="ExternalOutput")
    DR = mybir.MatmulPerfMode.DoubleRow

    with ExitStack() as ctx:
        tc = ctx.enter_context(tile.TileContext(nc))
        pool = ctx.enter_context(tc.tile_pool(name="main", bufs=1))
        psum = ctx.enter_context(
            tc.tile_pool(name="ps", bufs=1, space=bass.MemorySpace.PSUM))

        # HWDGE queues are SP and Act only. SP (fastest issue) takes 3 big
        # DMAs, Act takes 1 before its warm ops; Pool (SWDGE) the small ones.
        fts = [pool.tile([128, 8, RPC], FP8, name=f"ft{g}") for g in range(2)]
        wts = [pool.tile([128, 8, C], FP8, name=f"wt{g}") for g in range(2)]
        nc.sync.dma_start(fts[0][:], fT4[0, :, :, :])
        nc.sync.dma_start(wts[0][:], WT4[0, :, :, :])
        nc.sync.dma_start(fts[1][:], fT4[1, :, :, :])
        nc.scalar.dma_start(wts[1][:], WT4[1, :, :, :])
        warm = pool.tile([128, 1], F32)
        nc.gpsimd.memset(warm[:], 1.0)
        ob = pool.tile([1, RPC + C], BF16)
        nc.gpsimd.dma_start(ob[:], OB[:, :])
        ein = pool.tile([SRC_PC, PCAP], BF16)
        nc.gpsimd.dma_start(ein[:], EIN[:, :])

        # warm activation anchors the (single) act-table load early
        nc.scalar.activation(warm[:], warm[:], AF.Exp)

        yp = psum.tile([RPC, C], F32)
        for l in range(4):
            nc.tensor.matmul(yp[:], fts[0][:, 2 * l:2 * l + 2, :],
                             wts[0][:, 2 * l:2 * l + 2, :],
                             start=(l == 0), stop=False, perf_mode=DR)
        nc.tensor.matmul(yp[:], ob[:, 0:RPC], ob[:, RPC:RPC + C],
                         start=False, stop=False)
        for l in range(4):
            nc.tensor.matmul(yp[:], fts[1][:, 2 * l:2 * l + 2, :],
                             wts[1][:, 2 * l:2 * l + 2, :],
                             start=False, stop=(l == 3), perf_mode=DR)

        # tempered softmax numerators + row sums (critical path: before the
        # logits copy-out, which the tile tracker serializes in program order)
        et = pool.tile([RPC, C], BF16)
        z = pool.tile([RPC, 1], F32)
        nc.scalar.activation(et[:], yp[:], AF.Exp, scale=EXPS, accum_out=z[:])

        # logits copy (its DMA is emitted later, on the Act queue, so the SP
        # sequencer stays free for the chain-critical GO dma: a dma holds its
        # queue's SEQ until descriptor generation completes)
        yout = pool.tile([RPC, C], BF16)
        nc.vector.tensor_copy(yout[:], yp[:])
        rz = pool.tile([SRC_PC, 1], F32)
        nc.vector.reciprocal(rz[:], z[0:SRC_PC, :])
        ep = pool.tile([SRC_PC, PCAP], BF16)
        nc.vector.tensor_scalar_mul(ep[:], ein[:], rz[:])

        # U[p, c] = S_a + S_b for pair p = (a, b); pairs only use src slots
        psU = psum.tile([PCAP, C], F32)
        nc.tensor.matmul(psU[:], ep[:], et[0:SRC_PC, :], start=True, stop=True)
        lu = pool.tile([PCAP, C], F32)
        nc.scalar.activation(lu[:], psU[:], AF.Ln)
        junk = pool.tile([PCAP, C], BF16)
        g_out = pool.tile([PCAP, 1], F32)
        nc.vector.scalar_tensor_tensor(junk[:], psU[:], 0.0, lu[:],
                                       AL.bypass, AL.mult, accum_out=g_out[:])
        nc.sync.dma_start(GO[:, :], g_out[:])
        nc.scalar.dma_start(YO[:, :], yout[:])

    # Restrict the act-table pass to the one set serving BOTH Exp and Ln:
    # otherwise every Exp<->Ln switch emits a 1283ns table reload. The
    # act_func_set_id is positional (index into act_info.json), so keep all
    # entries but blank the funcs of every other set.
    real_get = bacc.get_activation_tables
    def only_combined(arch):
        tabs = real_get(arch)
        keep = "natural_log_exp_and_others"
        return {name: (funcs if name == keep else set())
                for name, funcs in tabs.items()}
    bacc.get_activation_tables = only_combined
    try:
        nc.compile()
    finally:
        bacc.get_activation_tables = real_get
    return nc


def _pack_classes(lab):
    """Assign source rows to cores by label class so ss pairs are core-local.

    Returns (src_rows[8][64], pairs[8] list of (slot_a, slot_b),
    spill list of (global_i, global_j))."""
    classes = {}
    for k in np.unique(lab):
        classes[int(k)] = np.nonzero(lab == k)[0]
    pair_cls = [(len(v) * (len(v) - 1) // 2, k)
                for k, v in classes.items() if len(v) >= 2]
    pair_cls.sort(reverse=True)
    bin_rows = [[] for _ in range(NCORES)]
    bin_cls = [[] for _ in range(NCORES)]
    bin_pairs = [0] * NCORES
    spill_cls = []
    for p, k in pair_cls:
        rows = classes[k]
        cand = [c for c in range(NCORES)
                if len(bin_rows[c]) + len(rows) <= SRC_PC
                and bin_pairs[c] + p <= PCAP]
        if cand:
            c = min(cand, key=lambda c: bin_pairs[c])
            bin_rows[c].extend(rows.tolist())
            bin_cls[c].append(k)
            bin_pairs[c] += p
        else:
            cand2 = [c for c in range(NCORES)
                     if len(bin_rows[c]) + len(rows) <= SRC_PC]
            if cand2:
                # rows co-located; on-device pairs up to capacity, rest spill
                c = min(cand2, key=lambda c: bin_pairs[c])
                bin_rows[c].extend(rows.tolist())
                bin_cls[c].append((k, PCAP - bin_pairs[c]))
                bin_pairs[c] = PCAP
            else:
                spill_cls.append(k)  # whole class on host
    # leftover rows (singletons, spilled classes) fill remaining slots
    used = set()
    for c in range(NCORES):
        used.update(bin_rows[c])
    leftover = [i for i in range(len(lab)) if i not in used]
    li = 0
    for c in range(NCORES):
        while len(bin_rows[c]) < SRC_PC:
            bin_rows[c].append(leftover[li])
            li += 1
    assert li == len(leftover)

    # build local pair lists
    spill = []
    pairs = [[] for _ in range(NCORES)]
    for c in range(NCORES):
        slot_of = {g: s for s, g in enumerate(bin_rows[c])}
        for entry in bin_cls[c]:
            if isinstance(entry, tuple):
                k, cap = entry
            else:
                k, cap = entry, None
            rows = classes[k]
            cnt = 0
            for a in range(len(rows)):
                for b2 in range(a + 1, len(rows)):
                    if cap is not None and cnt >= cap:
                        spill.append((rows[a], rows[b2]))
                    else:
                        pairs[c].append((slot_of[rows[a]], slot_of[rows[b2]]))
                    cnt += 1
    for k in spill_cls:
        rows = classes[k]
        for a in range(len(rows)):
            for b2 in range(a + 1, len(rows)):
                spill.append((rows[a], rows[b2]))
    return bin_rows, pairs, spill


def _pack_ft(m):
    """[rows, K] fp8 row-block -> [2, 128, 8, rows] with 1KB-contiguous
    per-partition lines (8 contraction chunks packed per descriptor)."""
    r = m.shape[0]
    arr = np.ascontiguousarray(m.T).reshape(16, 128, r)      # [chunk, p, r]
    return np.ascontiguousarray(
        arr.reshape(2, 8, 128, r).transpose(0, 2, 1, 3))     # [g, p, l, r]


def kernel(f, W, b, labels_s, _trace=False, _timings=None):
    f = np.asarray(f, dtype=np.float32)
    W = np.asarray(W, dtype=np.float32)
    b = np.asarray(b, dtype=np.float32)
    labels = np.asarray(labels_s)
    lab = labels[:BS]

    if "fused" not in _cache:
        _cache["fused"] = _build_fused()
    nc = _cache["fused"]

    # ---- host: class->core packing and input layout ----
    bin_rows, pairs, spill = _pack_classes(lab)
    fq = f.astype(NP_FP8)
    Wq = (W * SC).astype(NP_FP8)
    WT4 = _pack_ft(Wq)
    ob = np.concatenate([np.ones(RPC, np.float32),
                         SC * b]).reshape(1, RPC + C).astype(NP_BF16)

    core_rows = []
    in_maps = []
    for c in range(NCORES):
        rows = list(bin_rows[c]) + list(range(BS + c * TGT_PC,
                                              BS + (c + 1) * TGT_PC))
        core_rows.append(rows)
        E = np.zeros((SRC_PC, PCAP), np.float32)
        for p, (a, b2) in enumerate(pairs[c]):
            E[a, p] += 1.0
            E[b2, p] += 1.0
        for p in range(len(pairs[c]), PCAP):
            E[0, p] = 2.0  # dummy pair -> finite G, ignored by host
        in_maps.append({
            "fT4": _pack_ft(fq[rows]),
            "WT4": WT4,
            "EIN": E.astype(NP_BF16),
            "OB": ob,
        })

    r = run_bass_kernel_spmd(nc, in_maps, core_ids=list(range(NCORES)),
                             trace=_trace)
    if _timings is not None:
        _timings.append(("fused", r.exec_time_ns))

    # ---- host: unpermute logits, softmax stats ----
    rawpp = np.empty((N, C), np.float64)
    for c in range(NCORES):
        rawpp[core_rows[c]] = np.asarray(
            r.results[c]["YO"]).astype(np.float64)
    y = rawpp / (2.0 * SC)              # == (f@W.T + b)/2
    y_t = y[BS:]
    pseudo = np.argmax(y_t, 1)
    e2 = np.exp(y_t - y_t.max(1, keepdims=True))
    conf = (e2 / e2.sum(1, keepdims=True))[np.arange(BS), pseudo]
    yt2 = y / 2.0
    eS = np.exp(yt2 - yt2.max(1, keepdims=True))
    S = eS / eS.sum(1, keepdims=True)
    H = (S * np.log(S)).sum(1)

    # ---- ss loss: device G + host spill ----
    ss_sum = 0.0
    ss_cnt = 0
    for c in range(NCORES):
        gvals = np.asarray(r.results[c]["GO"]).reshape(-1).astype(np.float64)
        rows = core_rows[c]
        for p, (a, b2) in enumerate(pairs[c]):
            ga, gb = rows[a], rows[b2]
            ss_sum += 0.5 * (H[ga] + H[gb]) + LN2 - 0.5 * gvals[p]
            ss_cnt += 1
    for (ga, gb) in spill:
        u = S[ga] + S[gb]
        ss_sum += 0.5 * (H[ga] + H[gb]) + LN2 - 0.5 * (u * np.log(u)).sum()
        ss_cnt += 1
    loss_ss = ss_sum / ss_cnt if ss_cnt else 0.0

    # ---- st loss fully on host (tiny, data-dependent mask) ----
    passing = np.nonzero(conf >= THRESHOLD)[0]
    st_sum = 0.0
    st_cnt = 0
    for j in passing:
        gj = BS + j
        for gi in np.nonzero(lab == pseudo[j])[0]:
            u = S[gi] + S[gj]
            st_sum += 0.5 * (H[gi] + H[gj]) + LN2 - 0.5 * (u * np.log(u)).sum()
            st_cnt += 1
    loss_st = st_sum / st_cnt if st_cnt else 0.0

    loss = np.float32(4.0 * (loss_ss + loss_st))
    return (loss, np.float32(0.0))


# revision 22
# speedup vs baseline: 1.0453x; 1.0453x over previous
"""Trainium2 Bass kernel for nn_AdversarialLoss_PDD (pairwise JS-divergence loss).

Single fused kernel. Math (validated vs reference in fp64):
  raw = f @ W.T + b, y = raw/2, Ss/St = softmax(raw/4),
  H_i = sum_c S ln S, JS[i,j] = 0.5(H_i+H_j) + ln2 - 0.5*G[i,j],
  G[i,j] = sum_c (S_i+S_j) ln(S_i+S_j).

Only same-label (ss) and label==pseudo&conf (st) pairs contribute. The ss
pair list depends only on labels (known before launch), so rows are
assigned to cores BY CLASS: each core gets 64 source rows (same-label
groups co-located) + 64 target rows.  One kernel per core then:
  1. logits raw'' = fp8(f) @ fp8(W*sqrt(K)).T + sqrt(K)*b
     (8 fp8 DoubleRow matmuls, 2 contraction chunks each, + 1 bias outer)
  2. ET = exp(raw''/(4*sqrt(K))) bf16, z = rowsum, rz = 1/z
  3. U = matmul(E*rz, ET): one-hot pair-selection matrix E (host input)
     gives U[p,c] = S_a + S_b for pair p's rows (a,b)
  4. G[p] = sum_c U ln U  via ACT Ln + DVE mult-accum
Outputs: raw'' (bf16) and G (f32). Host computes softmax stats/H/conf/
pseudo from raw'', the ~35 st pairs + spilled ss pairs, masked means.
End-to-end loss rel err vs fp64 reference (on HW): 1.9e-3 (tol 2e-2).

Timing (CoreSim cost model, single launch): 7779 ns vs 20894 ns for the
previous two-phase windowed kernel (2.7x).  Remaining time is structural:
~2950 input DMA+matmul front (bus-bound at 0.53MB/core fp8), ~1900 the
exp->z->1/z->E'->matmul->ln->mult chain (sem-prop dominated), ~2200 the
final G DMA (seq+DGE+sem fixed costs), ~600 drain.  Tried and rejected:
DVE divide (not lowerable), partition-split pipelining (engine time is
free-dim bound), act-table tricks beyond the single-table pin, DMA
queue/group reshuffles (bus-bound floor reached).
"""

import math
import sys
import numpy as np
from contextlib import ExitStack

for _p in ("/opt/trn_rl_repo", "/root/.axon_site/_ro/trn_rl_repo"):
    if _p not in sys.path:
        sys.path.append(_p)

import ml_dtypes
import concourse.bass as bass
import concourse.tile as tile
from concourse import bacc, mybir
from concourse.bass_utils import run_bass_kernel_spmd

F32 = mybir.dt.float32
BF16 = mybir.dt.bfloat16
FP8 = mybir.dt.float8e4
AL = mybir.AluOpType
AF = mybir.ActivationFunctionType
NP_FP8 = ml_dtypes.float8_e4m3
NP_BF16 = ml_dtypes.bfloat16

NCORES = 8
C = 128            # n classes
K = 2048           # in features
N = 1024           # batch (source+target)
BS = 512           # source rows
SRC_PC = BS // NCORES   # 64 source slots per core
TGT_PC = BS // NCORES   # 64 target slots per core
RPC = SRC_PC + TGT_PC   # 128 rows per core
PCAP = 128              # pair columns per core (partition-limited)

THRESHOLD = 0.05
LN2 = math.log(2.0)
SC = math.sqrt(float(K))         # f8 weight pre-scale
EXPS = 0.25 / SC                 # device exp scale for tempered softmax

_cache = {}


def _build_fused():
    """Per core: 128-row logits (fp8 DoubleRow matmul) + per-pair G."""
    nc = bacc.Bacc(None, target_bir_lowering=False)
    fT4 = nc.dram_tensor("fT4", [2, 128, 8, RPC], FP8, kind="ExternalInput")
    WT4 = nc.dram_tensor("WT4", [2, 128, 8, C], FP8, kind="ExternalInput")
    EIN = nc.dram_tensor("EIN", [SRC_PC, PCAP], BF16, kind="ExternalInput")
    OB = nc.dram_tensor("OB", [1, RPC + C], BF16, kind="ExternalInput")
    YO = nc.dram_tensor("YO", [RPC, C], BF16, kind="ExternalOutput")
    GO = nc.dram_tensor("GO", [PCAP, 1], F32, kind="ExternalOutput")
    DR = mybir.MatmulPerfMode.DoubleRow

    with ExitStack() as ctx:
        tc = ctx.enter_context(tile.TileContext(nc))
        pool = ctx.enter_context(tc.tile_pool(name="main", bufs=1))
        psum = ctx.enter_context(
            tc.tile_pool(name="ps", bufs=1, space=bass.MemorySpace.PSUM))

        # HWDGE queues are SP and Act only. SP (fastest issue) takes 3 big
        # DMAs, Act takes 1 before its warm ops; Pool (SWDGE) the small ones.
        fts = [pool.tile([128, 8, RPC], FP8, name=f"ft{g}") for g in range(2)]
        wts = [pool.tile([128, 8, C], FP8, name=f"wt{g}") for g in range(2)]
        nc.sync.dma_start(fts[0][:], fT4[0, :, :, :])
        nc.sync.dma_start(wts[0][:], WT4[0, :, :, :])
        nc.sync.dma_start(fts[1][:], fT4[1, :, :, :])
        nc.scalar.dma_start(wts[1][:], WT4[1, :, :, :])
        warm = pool.tile([128, 1], F32)
        nc.gpsimd.memset(warm[:], 1.0)
        ob = pool.tile([1, RPC + C], BF16)
        nc.gpsimd.dma_start(ob[:], OB[:, :])
        ein = pool.tile([SRC_PC, PCAP], BF16)
        nc.gpsimd.dma_start(ein[:], EIN[:, :])

        # warm activation anchors the (single) act-table load early
        nc.scalar.activation(warm[:], warm[:], AF.Exp)

        yp = psum.tile([RPC, C], F32)
        for l in range(4):
            nc.tensor.matmul(yp[:], fts[0][:, 2 * l:2 * l + 2, :],
                             wts[0][:, 2 * l:2 * l + 2, :],
                             start=(l == 0), stop=False, perf_mode=DR)
        nc.tensor.matmul(yp[:], ob[:, 0:RPC], ob[:, RPC:RPC + C],
                         start=False, stop=False)
        for l in range(4):
            nc.tensor.matmul(yp[:], fts[1][:, 2 * l:2 * l + 2, :],
                             wts[1][:, 2 * l:2 * l + 2, :],
                             start=False, stop=(l == 3), perf_mode=DR)

        # tempered softmax numerators + row sums (critical path: before the
        # logits copy-out, which the tile tracker serializes in program order)
        et = pool.tile([RPC, C], BF16)
        z = pool.tile([RPC, 1], F32)
        nc.scalar.activation(et[:], yp[:], AF.Exp, scale=EXPS, accum_out=z[:])

        # logits copy (its DMA is emitted later, on the Act queue, so the SP
        # sequencer stays free for the chain-critical GO dma: a dma holds its
        # queue's SEQ until descriptor generation completes)
        yout = pool.tile([RPC, C], BF16)
        nc.vector.tensor_copy(yout[:], yp[:])
        rz = pool.tile([SRC_PC, 1], F32)
        nc.vector.reciprocal(rz[:], z[0:SRC_PC, :])
        ep = pool.tile([SRC_PC, PCAP], BF16)
        nc.vector.tensor_scalar_mul(ep[:], ein[:], rz[:])

        # U[p, c] = S_a + S_b for pair p = (a, b); pairs only use src slots
        psU = psum.tile([PCAP, C], F32)
        nc.tensor.matmul(psU[:], ep[:], et[0:SRC_PC, :], start=True, stop=True)
        lu = pool.tile([PCAP, C], F32)
        nc.scalar.activation(lu[:], psU[:], AF.Ln)
        junk = pool.tile([PCAP, C], BF16)
        g_out = pool.tile([PCAP, 1], F32)
        nc.vector.scalar_tensor_tensor(junk[:], psU[:], 0.0, lu[:],
                                       AL.bypass, AL.mult, accum_out=g_out[:])
        nc.sync.dma_start(GO[:, :], g_out[:])
        nc.gpsimd.dma_start(YO[:, :], yout[:])

    # Restrict the act-table pass to the one set serving BOTH Exp and Ln:
    # otherwise every Exp<->Ln switch emits a 1283ns table reload. The
    # act_func_set_id is positional (index into act_info.json), so keep all
    # entries but blank the funcs of every other set.
    real_get = bacc.get_activation_tables
    def only_combined(arch):
        tabs = real_get(arch)
        keep = "natural_log_exp_and_others"
        return {name: (funcs if name == keep else set())
                for name, funcs in tabs.items()}
    bacc.get_activation_tables = only_combined
    try:
        nc.compile()
    finally:
        bacc.get_activation_tables = real_get
    return nc


def _pack_classes(lab):
    """Assign source rows to cores by label class so ss pairs are core-local.

    Returns (src_rows[8][64], pairs[8] list of (slot_a, slot_b),
    spill list of (global_i, global_j))."""
    classes = {}
    for k in np.unique(lab):
        classes[int(k)] = np.nonzero(lab == k)[0]
    pair_cls = [(len(v) * (len(v) - 1) // 2, k)
                for k, v in classes.items() if len(v) >= 2]
    pair_cls.sort(reverse=True)
    bin_rows = [[] for _ in range(NCORES)]
    bin_cls = [[] for _ in range(NCORES)]
    bin_pairs = [0] * NCORES
    spill_cls = []
    for p, k in pair_cls:
        rows = classes[k]
        cand = [c for c in range(NCORES)
                if len(bin_rows[c]) + len(rows) <= SRC_PC
                and bin_pairs[c] + p <= PCAP]
        if cand:
            c = min(cand, key=lambda c: bin_pairs[c])
            bin_rows[c].extend(rows.tolist())
            bin_cls[c].append(k)
            bin_pairs[c] += p
        else:
            cand2 = [c for c in range(NCORES)
                     if len(bin_rows[c]) + len(rows) <= SRC_PC]
            if cand2:
                # rows co-located; on-device pairs up to capacity, rest spill
                c = min(cand2, key=lambda c: bin_pairs[c])
                bin_rows[c].extend(rows.tolist())
                bin_cls[c].append((k, PCAP - bin_pairs[c]))
                bin_pairs[c] = PCAP
            else:
                spill_cls.append(k)  # whole class on host
    # leftover rows (singletons, spilled classes) fill remaining slots
    used = set()
    for c in range(NCORES):
        used.update(bin_rows[c])
    leftover = [i for i in range(len(lab)) if i not in used]
    li = 0
    for c in range(NCORES):
        while len(bin_rows[c]) < SRC_PC:
            bin_rows[c].append(leftover[li])
            li += 1
    assert li == len(leftover)

    # build local pair lists
    spill = []
    pairs = [[] for _ in range(NCORES)]
    for c in range(NCORES):
        slot_of = {g: s for s, g in enumerate(bin_rows[c])}
        for entry in bin_cls[c]:
            if isinstance(entry, tuple):
                k, cap = entry
            else:
                k, cap = entry, None
            rows = classes[k]
            cnt = 0
            for a in range(len(rows)):
                for b2 in range(a + 1, len(rows)):
                    if cap is not None and cnt >= cap:
                        spill.append((rows[a], rows[b2]))
                    else:
                        pairs[c].append((slot_of[rows[a]], slot_of[rows[b2]]))
                    cnt += 1
    for k in spill_cls:
        rows = classes[k]
        for a in range(len(rows)):
            for b2 in range(a + 1, len(rows)):
                spill.append((rows[a], rows[b2]))
    return bin_rows, pairs, spill


def _pack_ft(m):
    """[rows, K] fp8 row-block -> [2, 128, 8, rows] with 1KB-contiguous
    per-partition lines (8 contraction chunks packed per descriptor)."""
    r = m.shape[0]
    arr = np.ascontiguousarray(m.T).reshape(16, 128, r)      # [chunk, p, r]
    return np.ascontiguousarray(
        arr.reshape(2, 8, 128, r).transpose(0, 2, 1, 3))     # [g, p, l, r]


def kernel(f, W, b, labels_s, _trace=False, _timings=None):
    f = np.asarray(f, dtype=np.float32)
    W = np.asarray(W, dtype=np.float32)
    b = np.asarray(b, dtype=np.float32)
    labels = np.asarray(labels_s)
    lab = labels[:BS]

    if "fused" not in _cache:
        _cache["fused"] = _build_fused()
    nc = _cache["fused"]

    # ---- host: class->core packing and input layout ----
    bin_rows, pairs, spill = _pack_classes(lab)
    fq = f.astype(NP_FP8)
    Wq = (W * SC).astype(NP_FP8)
    WT4 = _pack_ft(Wq)
    ob = np.concatenate([np.ones(RPC, np.float32),
                         SC * b]).reshape(1, RPC + C).astype(NP_BF16)

    core_rows = []
    in_maps = []
    for c in range(NCORES):
        rows = list(bin_rows[c]) + list(range(BS + c * TGT_PC,
                                              BS + (c + 1) * TGT_PC))
        core_rows.append(rows)
        E = np.zeros((SRC_PC, PCAP), np.float32)
        for p, (a, b2) in enumerate(pairs[c]):
            E[a, p] += 1.0
            E[b2, p] += 1.0
        for p in range(len(pairs[c]), PCAP):
            E[0, p] = 2.0  # dummy pair -> finite G, ignored by host
        in_maps.append({
            "fT4": _pack_ft(fq[rows]),
            "WT4": WT4,
            "EIN": E.astype(NP_BF16),
            "OB": ob,
        })

    r = run_bass_kernel_spmd(nc, in_maps, core_ids=list(range(NCORES)),
                             trace=_trace)
    if _timings is not None:
        _timings.append(("fused", r.exec_time_ns))

    # ---- host: unpermute logits, softmax stats ----
    rawpp = np.empty((N, C), np.float64)
    for c in range(NCORES):
        rawpp[core_rows[c]] = np.asarray(
            r.results[c]["YO"]).astype(np.float64)
    y = rawpp / (2.0 * SC)              # == (f@W.T + b)/2
    y_t = y[BS:]
    pseudo = np.argmax(y_t, 1)
    e2 = np.exp(y_t - y_t.max(1, keepdims=True))
    conf = (e2 / e2.sum(1, keepdims=True))[np.arange(BS), pseudo]
    yt2 = y / 2.0
    eS = np.exp(yt2 - yt2.max(1, keepdims=True))
    S = eS / eS.sum(1, keepdims=True)
    H = (S * np.log(S)).sum(1)

    # ---- ss loss: device G + host spill ----
    ss_sum = 0.0
    ss_cnt = 0
    for c in range(NCORES):
        gvals = np.asarray(r.results[c]["GO"]).reshape(-1).astype(np.float64)
        rows = core_rows[c]
        for p, (a, b2) in enumerate(pairs[c]):
            ga, gb = rows[a], rows[b2]
            ss_sum += 0.5 * (H[ga] + H[gb]) + LN2 - 0.5 * gvals[p]
            ss_cnt += 1
    for (ga, gb) in spill:
        u = S[ga] + S[gb]
        ss_sum += 0.5 * (H[ga] + H[gb]) + LN2 - 0.5 * (u * np.log(u)).sum()
        ss_cnt += 1
    loss_ss = ss_sum / ss_cnt if ss_cnt else 0.0

    # ---- st loss fully on host (tiny, data-dependent mask) ----
    passing = np.nonzero(conf >= THRESHOLD)[0]
    st_sum = 0.0
    st_cnt = 0
    for j in passing:
        gj = BS + j
        for gi in np.nonzero(lab == pseudo[j])[0]:
            u = S[gi] + S[gj]
            st_sum += 0.5 * (H[gi] + H[gj]) + LN2 - 0.5 * (u * np.log(u)).sum()
            st_cnt += 1
    loss_st = st_sum / st_cnt if st_cnt else 0.0

    loss = np.float32(4.0 * (loss_ss + loss_st))
    return (loss, np.float32(0.0))
